# revision 3
# baseline (speedup 1.0000x reference)
"""DPSOM forward kernel for Trainium2, data-parallel over 8 NeuronCores.

Math (per sample row x):
    h  = relu(x @ We1 + be1)            [784] -> [256]
    z  = h @ We2 + be2                  [256] -> [32]
    j  = argmin_j ||z - P_j||^2         (25 prototypes)
    som_z = P_j
    x_recon = sigmoid(relu(som_z @ Wd1 + bd1) @ Wd2 + bd2)

Key structural facts exploited:
  * som_z takes only 25 distinct values, so the whole decoder collapses to a
    25-row lookup table computed once on the host; on device the decoder is a
    one-hot [B,25] x table [25,784] matmul (exact row selection).  The table
    is split into bf16 hi/lo parts so the selection runs at full PE rate and
    still reconstructs fp32 values to ~1e-5.
  * Everything is computed in a batch-transposed layout (batch along the free
    axis) so the encoder weights stay as natural-layout lhsT tiles; x is
    transposed host-side per shard.  The encoder matmuls run in plain fp32 so
    z (and hence the argmin) match the fp32 reference.
  * z is emitted in transposed layout [32, B] and transposed back on host.
"""

import numpy as np
import ml_dtypes
from contextlib import ExitStack

import concourse.bass as bass
import concourse.tile as tile
from concourse import bacc, mybir
from concourse import bass_utils

F32 = mybir.dt.float32
BF16 = mybir.dt.bfloat16
ALU = mybir.AluOpType
ACTF = mybir.ActivationFunctionType
AX = mybir.AxisListType

B, D, H, L, NP = 65536, 784, 256, 32, 25
NCORES = 8
BC = B // NCORES            # batch per core
MEGA = 1024                 # batch columns processed per outer step
KT = [(0, 128), (128, 128), (256, 128), (384, 128), (512, 128), (640, 128), (768, 16)]
BIG = float(2.0 ** 30)


def build_nc(bc=BC, mega=MEGA):
    nmega = bc // mega
    nchunk = mega // 128
    nc = bacc.Bacc("TRN2", target_bir_lowering=False, debug=False, enable_asserts=False)

    xT = nc.dram_tensor("xT", [D, bc], F32, kind="ExternalInput").ap()
    we1 = nc.dram_tensor("We1", [D, H], F32, kind="ExternalInput").ap()
    we2 = nc.dram_tensor("We2", [H, L], F32, kind="ExternalInput").ap()
    be1 = nc.dram_tensor("be1", [H, 1], F32, kind="ExternalInput").ap()
    be2 = nc.dram_tensor("be2", [L, 1], F32, kind="ExternalInput").ap()
    pd = nc.dram_tensor("pd", [L + 1, NP], F32, kind="ExternalInput").ap()
    ph = nc.dram_tensor("p_h", [NP, L], BF16, kind="ExternalInput").ap()
    pl = nc.dram_tensor("p_l", [NP, L], BF16, kind="ExternalInput").ap()
    tabh = nc.dram_tensor("tab_h", [NP, D], BF16, kind="ExternalInput").ap()
    tabl = nc.dram_tensor("tab_l", [NP, D], BF16, kind="ExternalInput").ap()
    iota = nc.dram_tensor("iota", [128, NP], F32, kind="ExternalInput").ap()
    eye = nc.dram_tensor("eye16", [128, 128], BF16, kind="ExternalInput").ap()
    xr = nc.dram_tensor("x_recon", [bc, D], F32, kind="ExternalOutput").ap()
    zo = nc.dram_tensor("z_outT", [L, bc], F32, kind="ExternalOutput").ap()
    szo = nc.dram_tensor("som_z", [bc, L], F32, kind="ExternalOutput").ap()

    with tile.TileContext(nc) as tc:
        with ExitStack() as ctx:
            cpool = ctx.enter_context(tc.tile_pool(name="consts", bufs=1))
            xpool = ctx.enter_context(tc.tile_pool(name="xt", bufs=8))
            hpool = ctx.enter_context(tc.tile_pool(name="ht", bufs=4))
            ztpool = ctx.enter_context(tc.tile_pool(name="zt", bufs=2))
            smpool = ctx.enter_context(tc.tile_pool(name="small", bufs=3))
            ohpool = ctx.enter_context(tc.tile_pool(name="oh", bufs=3))
            stpool = ctx.enter_context(tc.tile_pool(name="stage", bufs=2))
            php = ctx.enter_context(tc.tile_pool(name="ph", bufs=2, space="PSUM"))
            pzp = ctx.enter_context(tc.tile_pool(name="pz", bufs=1, space="PSUM"))
            psp = ctx.enter_context(tc.tile_pool(name="ps", bufs=3, space="PSUM"))
            prp = ctx.enter_context(tc.tile_pool(name="pr", bufs=2, space="PSUM"))

            we1_sb = cpool.tile([128, 7 * H], F32)
            for ki, (k0, ks) in enumerate(KT):
                nc.sync.dma_start(we1_sb[:ks, ki * H:(ki + 1) * H], we1[k0:k0 + ks, :])
            we2_sb = cpool.tile([128, 2 * L], F32)
            for k2 in range(2):
                nc.sync.dma_start(we2_sb[:, k2 * L:(k2 + 1) * L], we2[k2 * 128:(k2 + 1) * 128, :])
            be1_sb = cpool.tile([128, 2], F32)
            for m in range(2):
                nc.sync.dma_start(be1_sb[:, m:m + 1], be1[m * 128:(m + 1) * 128, :])
            be2_sb = cpool.tile([L, 1], F32)
            nc.sync.dma_start(be2_sb[:, :], be2[:, :])
            pd_sb = cpool.tile([L + 1, NP], F32)
            nc.sync.dma_start(pd_sb[:, :], pd[:, :])
            ph_sb = cpool.tile([NP, L], BF16)
            nc.sync.dma_start(ph_sb[:, :], ph[:, :])
            pl_sb = cpool.tile([NP, L], BF16)
            nc.sync.dma_start(pl_sb[:, :], pl[:, :])
            tabh_sb = cpool.tile([NP, D], BF16)
            nc.sync.dma_start(tabh_sb[:, :], tabh[:, :])
            tabl_sb = cpool.tile([NP, D], BF16)
            nc.sync.dma_start(tabl_sb[:, :], tabl[:, :])
            iota_sb = cpool.tile([128, NP], F32)
            nc.sync.dma_start(iota_sb[:, :], iota[:, :])
            eye_sb = cpool.tile([128, 128], BF16)
            nc.sync.dma_start(eye_sb[:, :], eye[:, :])

            for g in range(nmega):
                b0 = g * mega
                xts = []
                for ki, (k0, ks) in enumerate(KT):
                    t = xpool.tile([128, mega], F32, tag="xt")
                    nc.sync.dma_start(t[:ks, :], xT[k0:k0 + ks, b0:b0 + mega])
                    xts.append(t)

                # encoder layer 1: hT[256, mega] = relu(We1.T @ xT + be1)
                hts = []
                for m in range(2):
                    ht = hpool.tile([128, mega], F32, tag="ht")
                    for nb in range(mega // 512):
                        hp = php.tile([128, 512], F32, tag="hp")
                        for ki, (k0, ks) in enumerate(KT):
                            nc.tensor.matmul(
                                hp[:, :],
                                we1_sb[:ks, ki * H + m * 128: ki * H + (m + 1) * 128],
                                xts[ki][:ks, nb * 512:(nb + 1) * 512],
                                start=(ki == 0), stop=(ki == len(KT) - 1),
                            )
                        nc.scalar.activation(
                            ht[:, nb * 512:(nb + 1) * 512], hp[:, :], ACTF.Relu,
                            bias=be1_sb[:, m:m + 1], scale=1.0,
                        )
                    hts.append(ht)

                # encoder layer 2 (transposed): zT[32, mega] = We2.T @ hT + be2
                # row 32 of zt is constant 1.0 (pairs with the ||p||^2 row of pd)
                zt = ztpool.tile([L + 1, mega], F32, tag="zt")
                nc.vector.memset(zt[L:L + 1, :], 1.0)
                for nb in range(mega // 512):
                    zp = pzp.tile([L, 512], F32, tag="zp")
                    for k2 in range(2):
                        nc.tensor.matmul(
                            zp[:, :],
                            we2_sb[:, k2 * L:(k2 + 1) * L],
                            hts[k2][:, nb * 512:(nb + 1) * 512],
                            start=(k2 == 0), stop=(k2 == 1),
                        )
                    nc.vector.tensor_scalar(
                        zt[0:L, nb * 512:(nb + 1) * 512], zp[:, :],
                        be2_sb[:, 0:1], None, op0=ALU.add,
                    )
                nc.sync.dma_start(zo[:, b0:b0 + mega], zt[0:L, :])

                xr_st = stpool.tile([128, nchunk, D], F32, tag="xr")
                sz_st = stpool.tile([128, nchunk, L], F32, tag="ss")

                for c in range(nchunk):
                    cs = c * 128
                    # dists[128, 25] = -2 z.P^T + ||p||^2  (+ const ||z||^2 omitted)
                    dp = psp.tile([128, NP], F32, tag="sm")
                    nc.tensor.matmul(
                        dp[:, :], zt[:, cs:cs + 128], pd_sb[:, :],
                        start=True, stop=True,
                    )
                    # argmin over the 25 free-axis entries, first-min tie-break
                    dmin = smpool.tile([128, 1], F32, tag="dmin")
                    nc.vector.tensor_reduce(dmin[:, :], dp[:, :], axis=AX.X, op=ALU.min)
                    keys = smpool.tile([128, NP], F32, tag="keys")
                    nc.vector.tensor_scalar(
                        keys[:, :], dp[:, :], dmin[:, 0:1], BIG,
                        op0=ALU.subtract, op1=ALU.mult,
                    )
                    keys2 = smpool.tile([128, NP], F32, tag="keys2")
                    idx = smpool.tile([128, 1], F32, tag="idx")
                    nc.vector.tensor_add(keys2[:, :], keys[:, :], iota_sb[:, :])
                    nc.vector.tensor_reduce(idx[:, :], keys2[:, :], axis=AX.X, op=ALU.min)
                    # one-hot in bf16 (exact 0/1)
                    oh = ohpool.tile([128, NP], BF16, tag="oh")
                    nc.vector.tensor_scalar(
                        oh[:, :], iota_sb[:, :], idx[:, 0:1], None, op0=ALU.is_equal,
                    )
                    # transpose one-hot to [25, 128] for use as lhsT
                    ohtp = psp.tile([NP, 128], BF16, tag="sm")
                    nc.tensor.transpose(ohtp[:, :], oh[:, :], eye_sb[:, :])
                    oht = ohpool.tile([NP, 128], BF16, tag="oht")
                    nc.vector.tensor_copy(oht[:, :], ohtp[:, :])
                    # som_z[128, 32] = onehot @ (P_h + P_l)
                    szp = psp.tile([128, L], F32, tag="sm")
                    nc.tensor.matmul(szp[:, :], oht[:, :], ph_sb[:, :], start=True, stop=False)
                    nc.tensor.matmul(szp[:, :], oht[:, :], pl_sb[:, :], start=False, stop=True)
                    nc.vector.tensor_copy(sz_st[:, c, :], szp[:, :])
                    # x_recon[128, 784] = onehot @ (tab_h + tab_l) (exact row selection)
                    for hi, (n0, nn) in enumerate(((0, 392), (392, 392))):
                        rp = prp.tile([128, 392], F32, tag="rp")
                        nc.tensor.matmul(
                            rp[:, :], oht[:, :], tabh_sb[:, n0:n0 + nn], start=True, stop=False)
                        nc.tensor.matmul(
                            rp[:, :], oht[:, :], tabl_sb[:, n0:n0 + nn], start=False, stop=True)
                        if hi == 0:
                            nc.scalar.copy(xr_st[:, c, n0:n0 + nn], rp[:, :])
                        else:
                            nc.vector.tensor_copy(xr_st[:, c, n0:n0 + nn], rp[:, :])

                nc.sync.dma_start(
                    xr[b0:b0 + mega, :].rearrange("(c p) f -> p c f", p=128), xr_st[:, :, :])
                nc.sync.dma_start(
                    szo[b0:b0 + mega, :].rearrange("(c p) f -> p c f", p=128), sz_st[:, :, :])

    nc.compile()
    return nc


def host_prep(x, We1, be1, We2, be2, Wd1, bd1, Wd2, bd2, prototypes, bc=BC, ncores=NCORES):
    """Build per-core input maps (numpy only, layout prep + 25-row decoder table)."""
    bf16 = ml_dtypes.bfloat16
    x = np.asarray(x, np.float32)
    P = np.asarray(prototypes, np.float32)
    # decoder applied to the 25 prototypes -> lookup table
    hd = np.maximum(P @ np.asarray(Wd1, np.float32) + np.asarray(bd1, np.float32), 0.0)
    logits = hd @ np.asarray(Wd2, np.float32) + np.asarray(bd2, np.float32)
    tab = (1.0 / (1.0 + np.exp(-logits))).astype(np.float32)
    tab_h = tab.astype(bf16)
    tab_l = (tab - tab_h.astype(np.float32)).astype(bf16)
    p_h = P.astype(bf16)
    p_l = (P - p_h.astype(np.float32)).astype(bf16)
    pdm = np.concatenate([-2.0 * P.T, np.sum(P * P, axis=1)[None, :]], axis=0).astype(np.float32)
    consts = {
        "We1": np.ascontiguousarray(We1, np.float32),
        "We2": np.ascontiguousarray(We2, np.float32),
        "be1": np.asarray(be1, np.float32).reshape(H, 1).copy(),
        "be2": np.asarray(be2, np.float32).reshape(L, 1).copy(),
        "pd": np.ascontiguousarray(pdm),
        "p_h": np.ascontiguousarray(p_h),
        "p_l": np.ascontiguousarray(p_l),
        "tab_h": np.ascontiguousarray(tab_h),
        "tab_l": np.ascontiguousarray(tab_l),
        "iota": np.tile(np.arange(NP, dtype=np.float32), (128, 1)),
        "eye16": np.eye(128, dtype=bf16),
    }
    in_maps = []
    for c in range(ncores):
        m = dict(consts)
        m["xT"] = np.ascontiguousarray(x[c * bc:(c + 1) * bc].T)
        in_maps.append(m)
    return in_maps


_CACHE = {}


def kernel(x, We1, be1, We2, be2, Wd1, bd1, Wd2, bd2, prototypes):
    if "nc" not in _CACHE:
        _CACHE["nc"] = build_nc()
    nc = _CACHE["nc"]
    in_maps = host_prep(x, We1, be1, We2, be2, Wd1, bd1, Wd2, bd2, prototypes)
    res = bass_utils.run_bass_kernel_spmd(nc, in_maps, core_ids=list(range(NCORES)))
    x_recon = np.concatenate([r["x_recon"] for r in res.results], axis=0)
    z = np.concatenate([np.ascontiguousarray(r["z_outT"].T) for r in res.results], axis=0)
    som_z = np.concatenate([r["som_z"] for r in res.results], axis=0)
    return x_recon, z, som_z


# revision 7
# speedup vs baseline: 12.8620x; 12.8620x over previous
"""DPSOM forward kernel for Trainium2, data-parallel over 8 NeuronCores.

Math (per sample row x):
    h  = relu(x @ We1 + be1)            [784] -> [256]
    z  = h @ We2 + be2                  [256] -> [32]
    j  = argmin_j ||z - P_j||^2         (25 prototypes)
    som_z = P_j
    x_recon = sigmoid(relu(som_z @ Wd1 + bd1) @ Wd2 + bd2)

Key structural facts exploited:
  * som_z takes only 25 distinct values, so the whole decoder collapses to a
    25-row lookup table computed once on the host; on device the decoder is a
    one-hot [B,25] x table [25,816] matmul (exact row selection).  P and the
    table are concatenated and split into bf16 hi/lo parts so the selection
    runs at full PE rate and still reconstructs fp32 values to ~1e-5.
  * Everything is computed in a batch-transposed layout (batch along the free
    axis) so the encoder weights stay as natural-layout lhsT tiles; x is
    transposed host-side per shard.  The encoder matmuls run in plain fp32 so
    z (and hence the argmin) matches the fp32 reference.
  * argmin via keys = (d - dmin) + j*2^-30: the true min row has key exactly
    j*2^-30 (d-dmin == 0), every other row is >= one fp32 ulp of d (~1e-6)
    larger, and ties resolve to the smallest j like the reference.
  * z is emitted in transposed layout [32, B] and transposed back on host.
"""

import numpy as np
import ml_dtypes
from contextlib import ExitStack

import concourse.bass as bass
import concourse.tile as tile
from concourse import bacc, mybir
from concourse import bass_utils

F32 = mybir.dt.float32
BF16 = mybir.dt.bfloat16
ALU = mybir.AluOpType
ACTF = mybir.ActivationFunctionType
AX = mybir.AxisListType

B, D, H, L, NP = 65536, 784, 256, 32, 25
NCORES = 8
BC = B // NCORES            # batch per core
MEGA = 1024                 # batch columns processed per outer step
KT = [(0, 128), (128, 128), (256, 128), (384, 128), (512, 128), (640, 128), (768, 16)]
SEL = L + D                 # fused [P | table] selection width = 816
NPD = 2 * 32 - 7            # 57: [onehot | zero-gap(7) | onehot] partition layout
IOTA_SCALE = float(2.0 ** -30)


def build_nc(bc=BC, mega=MEGA, out_eng="sync", sel_split=True, lookahead=1):
    nmega = bc // mega
    nchunk = mega // 128
    nc = bacc.Bacc("TRN2", target_bir_lowering=False, debug=False, enable_asserts=False)

    xT = nc.dram_tensor("xT", [D, bc], F32, kind="ExternalInput").ap()
    we1 = nc.dram_tensor("We1", [D, H], F32, kind="ExternalInput").ap()
    we2 = nc.dram_tensor("We2", [H, L], F32, kind="ExternalInput").ap()
    be1 = nc.dram_tensor("be1", [H, 1], F32, kind="ExternalInput").ap()
    be2 = nc.dram_tensor("be2", [L, 1], F32, kind="ExternalInput").ap()
    pd = nc.dram_tensor("pd", [L + 1, NP], F32, kind="ExternalInput").ap()
    ptc = nc.dram_tensor("ptc", [NPD, SEL], BF16, kind="ExternalInput").ap()
    iota = nc.dram_tensor("iota", [128, NPD], F32, kind="ExternalInput").ap()
    eye = nc.dram_tensor("eye16", [128, 128], BF16, kind="ExternalInput").ap()
    xr = nc.dram_tensor("x_recon", [bc, D], F32, kind="ExternalOutput").ap()
    zo = nc.dram_tensor("z_outT", [L, bc], F32, kind="ExternalOutput").ap()
    szo = nc.dram_tensor("som_z", [bc, L], F32, kind="ExternalOutput").ap()

    with tile.TileContext(nc) as tc:
        oeng = nc.scalar if out_eng == "scalar" else nc.sync
        with ExitStack() as ctx:
            cpool = ctx.enter_context(tc.tile_pool(name="consts", bufs=1))
            xpool = ctx.enter_context(tc.tile_pool(name="xt", bufs=15))
            hpool = ctx.enter_context(tc.tile_pool(name="ht", bufs=4))
            ztpool = ctx.enter_context(tc.tile_pool(name="zt", bufs=2))
            smpool = ctx.enter_context(tc.tile_pool(name="small", bufs=3))
            ohpool = ctx.enter_context(tc.tile_pool(name="oh", bufs=3))
            stpool = ctx.enter_context(tc.tile_pool(name="stage", bufs=2))
            php = ctx.enter_context(tc.tile_pool(name="php", bufs=2, space="PSUM"))
            pzp = ctx.enter_context(tc.tile_pool(name="pzp", bufs=1, space="PSUM"))
            pdp = ctx.enter_context(tc.tile_pool(name="pdp", bufs=2, space="PSUM"))
            ptp = ctx.enter_context(tc.tile_pool(name="ptp", bufs=1, space="PSUM"))
            psa = ctx.enter_context(tc.tile_pool(name="psa", bufs=1, space="PSUM"))
            psb = ctx.enter_context(tc.tile_pool(name="psb", bufs=1, space="PSUM"))

            we1_sb = cpool.tile([128, 7 * H], F32)
            for ki, (k0, ks) in enumerate(KT):
                nc.sync.dma_start(we1_sb[:ks, ki * H:(ki + 1) * H], we1[k0:k0 + ks, :])
            we2_sb = cpool.tile([128, 2 * L], F32)
            for k2 in range(2):
                nc.sync.dma_start(we2_sb[:, k2 * L:(k2 + 1) * L], we2[k2 * 128:(k2 + 1) * 128, :])
            be1_sb = cpool.tile([128, 2], F32)
            for m in range(2):
                nc.sync.dma_start(be1_sb[:, m:m + 1], be1[m * 128:(m + 1) * 128, :])
            be2_sb = cpool.tile([L, 1], F32)
            nc.sync.dma_start(be2_sb[:, :], be2[:, :])
            pd_sb = cpool.tile([L + 1, NP], F32)
            nc.sync.dma_start(pd_sb[:, :], pd[:, :])
            ptc_sb = cpool.tile([NPD, SEL], BF16)
            nc.sync.dma_start(ptc_sb[:, :], ptc[:, :])
            iota_sb = cpool.tile([128, NPD], F32)
            nc.sync.dma_start(iota_sb[:, :], iota[:, :])
            eye_sb = cpool.tile([128, 128], BF16)
            nc.sync.dma_start(eye_sb[:, :], eye[:, :])

            for g in range(nmega):
                b0 = g * mega
                xts = []
                for ki, (k0, ks) in enumerate(KT):
                    t = xpool.tile([128, mega], F32, tag="xt")
                    nc.sync.dma_start(t[:ks, :], xT[k0:k0 + ks, b0:b0 + mega])
                    xts.append(t)

                # encoder layer 1: hT[256, mega] = relu(We1.T @ xT + be1)
                hts = []
                for m in range(2):
                    ht = hpool.tile([128, mega], F32, tag="ht")
                    for nb in range(mega // 512):
                        hp = php.tile([128, 512], F32, tag="hp")
                        for ki, (k0, ks) in enumerate(KT):
                            nc.tensor.matmul(
                                hp[:, :],
                                we1_sb[:ks, ki * H + m * 128: ki * H + (m + 1) * 128],
                                xts[ki][:ks, nb * 512:(nb + 1) * 512],
                                start=(ki == 0), stop=(ki == len(KT) - 1),
                            )
                        nc.scalar.activation(
                            ht[:, nb * 512:(nb + 1) * 512], hp[:, :], ACTF.Relu,
                            bias=be1_sb[:, m:m + 1], scale=1.0,
                        )
                    hts.append(ht)

                # encoder layer 2 (transposed): zT[32, mega] = We2.T @ hT + be2
                # row 32 of zt is constant 1.0 (pairs with the ||p||^2 row of pd)
                zt = ztpool.tile([L + 1, mega], F32, tag="zt")
                nc.vector.memset(zt[L:L + 1, :], 1.0)
                for nb in range(mega // 512):
                    zp = pzp.tile([L, 512], F32, tag="zp")
                    for k2 in range(2):
                        nc.tensor.matmul(
                            zp[:, :],
                            we2_sb[:, k2 * L:(k2 + 1) * L],
                            hts[k2][:, nb * 512:(nb + 1) * 512],
                            start=(k2 == 0), stop=(k2 == 1),
                        )
                    nc.vector.tensor_scalar(
                        zt[0:L, nb * 512:(nb + 1) * 512], zp[:, :],
                        be2_sb[:, 0:1], None, op0=ALU.add,
                    )
                oeng.dma_start(zo[:, b0:b0 + mega], zt[0:L, :])

                xr_st = stpool.tile([128, nchunk, D], F32, tag="xr")
                sz_st = stpool.tile([128, nchunk, L], F32, tag="ss")

                def chunk_front(c):
                    # dists[128, 25] = -2 z.P^T + ||p||^2  (+ const ||z||^2 omitted)
                    dp_t = pdp.tile([128, NP], F32, tag="dp")
                    nc.tensor.matmul(
                        dp_t[:, :], zt[:, c * 128:(c + 1) * 128], pd_sb[:, :],
                        start=True, stop=True,
                    )
                    return dp_t

                def chunk_back(c, dp_t):
                    # argmin over 25 free-axis entries, first-min tie-break:
                    # keys = (d - dmin) + j*2^-30; min row keys == jmin*2^-30 exactly
                    dmin = smpool.tile([128, 1], F32, tag="dmin")
                    nc.vector.tensor_reduce(dmin[:, :], dp_t[:, :], axis=AX.X, op=ALU.min)
                    keys = smpool.tile([128, NP], F32, tag="keys")
                    nc.vector.scalar_tensor_tensor(
                        keys[:, :], dp_t[:, :], dmin[:, 0:1], iota_sb[:, 0:NP],
                        op0=ALU.subtract, op1=ALU.add,
                    )
                    idx = smpool.tile([128, 1], F32, tag="idx")
                    nc.vector.tensor_reduce(idx[:, :], keys[:, :], axis=AX.X, op=ALU.min)
                    # double one-hot [128, 57]: cols 0:25 and 32:57 both get the
                    # one-hot (iota repeats), cols 25:32 are a sentinel -> 0
                    oh = ohpool.tile([128, NPD], BF16, tag="oh")
                    nc.vector.tensor_scalar(
                        oh[:, :], iota_sb[:, :], idx[:, 0:1], None, op0=ALU.is_equal,
                    )
                    # transpose to [57, 128] for use as lhsT (hi rows 0:25, lo rows 32:57)
                    ohtp = ptp.tile([NPD, 128], BF16, tag="tp")
                    nc.tensor.transpose(ohtp[:, :], oh[:, :], eye_sb[:, :])
                    oht = ohpool.tile([NPD, 128], BF16, tag="oht")
                    nc.vector.tensor_copy(oht[:, :], ohtp[:, :])
                    # [som_z | x_recon][128, 816] = onehot @ ([P | tab]_h + [P | tab]_l)
                    sa = psa.tile([128, 512], F32, tag="sela")
                    sb = psb.tile([128, SEL - 512], F32, tag="selb")
                    nc.tensor.matmul(sa[:, :], oht[:, :], ptc_sb[:, 0:512],
                                     start=True, stop=True)
                    nc.tensor.matmul(sb[:, :], oht[:, :], ptc_sb[:, 512:SEL],
                                     start=True, stop=True)
                    nc.vector.tensor_copy(sz_st[:, c, :], sa[:, 0:L])
                    nc.scalar.copy(xr_st[:, c, 0:512 - L], sa[:, L:512])
                    nc.vector.tensor_copy(xr_st[:, c, 512 - L:D], sb[:, :])

                if lookahead:
                    prev = None
                    for c in range(nchunk):
                        dp_t = chunk_front(c)
                        if prev is not None:
                            chunk_back(*prev)
                        prev = (c, dp_t)
                    chunk_back(*prev)
                else:
                    for c in range(nchunk):
                        chunk_back(c, chunk_front(c))

                oeng.dma_start(
                    xr[b0:b0 + mega, :].rearrange("(c p) f -> p c f", p=128), xr_st[:, :, :])
                oeng.dma_start(
                    szo[b0:b0 + mega, :].rearrange("(c p) f -> p c f", p=128), sz_st[:, :, :])

    nc.compile()
    return nc


def host_prep(x, We1, be1, We2, be2, Wd1, bd1, Wd2, bd2, prototypes, bc=BC, ncores=NCORES):
    """Build per-core input maps (numpy only, layout prep + 25-row decoder table)."""
    bf16 = ml_dtypes.bfloat16
    x = np.asarray(x, np.float32)
    P = np.asarray(prototypes, np.float32)
    # decoder applied to the 25 prototypes -> lookup table
    hd = np.maximum(P @ np.asarray(Wd1, np.float32) + np.asarray(bd1, np.float32), 0.0)
    logits = hd @ np.asarray(Wd2, np.float32) + np.asarray(bd2, np.float32)
    tab = (1.0 / (1.0 + np.exp(-logits))).astype(np.float32)
    ptab = np.concatenate([P, tab], axis=1)          # [25, 816]
    pt_h = ptab.astype(bf16)
    pt_l = (ptab - pt_h.astype(np.float32)).astype(bf16)
    ptc = np.concatenate([pt_h, np.zeros((7, SEL), bf16), pt_l], axis=0)  # [57, 816]
    pdm = np.concatenate([-2.0 * P.T, np.sum(P * P, axis=1)[None, :]], axis=0).astype(np.float32)
    consts = {
        "We1": np.ascontiguousarray(We1, np.float32),
        "We2": np.ascontiguousarray(We2, np.float32),
        "be1": np.asarray(be1, np.float32).reshape(H, 1).copy(),
        "be2": np.asarray(be2, np.float32).reshape(L, 1).copy(),
        "pd": np.ascontiguousarray(pdm),
        "ptc": np.ascontiguousarray(ptc),
        "iota": np.tile(np.concatenate([
            np.arange(NP, dtype=np.float32) * np.float32(IOTA_SCALE),
            np.full(7, 1.0, np.float32),
            np.arange(NP, dtype=np.float32) * np.float32(IOTA_SCALE)]), (128, 1)),
        "eye16": np.eye(128, dtype=bf16),
    }
    in_maps = []
    for c in range(ncores):
        m = dict(consts)
        m["xT"] = np.ascontiguousarray(x[c * bc:(c + 1) * bc].T)
        in_maps.append(m)
    return in_maps


_CACHE = {}


def kernel(x, We1, be1, We2, be2, Wd1, bd1, Wd2, bd2, prototypes):
    if "nc" not in _CACHE:
        _CACHE["nc"] = build_nc()
    nc = _CACHE["nc"]
    in_maps = host_prep(x, We1, be1, We2, be2, Wd1, bd1, Wd2, bd2, prototypes)
    res = bass_utils.run_bass_kernel_spmd(nc, in_maps, core_ids=list(range(NCORES)))
    x_recon = np.concatenate([r["x_recon"] for r in res.results], axis=0)
    z = np.concatenate([np.ascontiguousarray(r["z_outT"].T) for r in res.results], axis=0)
    som_z = np.concatenate([r["som_z"] for r in res.results], axis=0)
    return x_recon, z, som_z
